# revision 1
# baseline (speedup 1.0000x reference)
"""DeepEMD Trainium2 kernel: batched 49x49 entropic-OT (Sinkhorn) similarity.

Strategy (8 NeuronCores, data-parallel over batch):
- Each core gets 128 batches. Host prepacks, per (chunk j of 128 channels,
  batch b), an augmented matrix A = [Q | P | 1] (128 x 99) in fp16 (10-bit
  mantissa keeps the end-to-end logits error ~2e-4), sequenced in DRAM so
  every load DMA reads one contiguous span.
- PE computes the Gram G_b = A^T A (99x99, fp32 PSUM) with one accumulating
  fp16 matmul per channel chunk (weights widened to 128 cols to engage
  fast-weight-load; junk rows ignored). G contains Q^T P, P^T Q, column
  sums (ones row) and diag blocks -> the similarity map, norms and weight
  vectors are all cheap fixups of G.
- A per-batch SBUF->SBUF DMA flattens G_b into row b of a [128, 99*99]
  tile: everything after that runs batch-on-partitions, full 128-lane DVE.
- Sinkhorn runs in the *linear* domain (K = exp((sim-1)/eps + 16)) with
  Gauss-Seidel updates us = r/(K vs), vs = c/(K^T us). The reference's 100
  log-domain iterations are converged ~1e-12 by 20; ITERS linear f32
  iterations reach ~2e-5 at 8.
- logits[b] = T * sum(flow * sim) = T * us^T ((K.sim) vs).
- One dma_start runs on a single SDMA engine (~27 GB/s), so loads are split
  into sub-DMAs across both HWDGE queues and flattens are spread across
  gpsimd/sync/scalar issuers to keep many engines streaming.
"""

import os
import sys

import numpy as np

sys.path.insert(0, "/opt/trn_rl_repo")

import concourse.bass as bass
import concourse.bacc as bacc
import concourse.mybir as mybir
from concourse import tile
from concourse.bass_utils import run_bass_kernel_spmd

import ml_dtypes

B_FULL, C, HW = 1024, 512, 49
NCORE = 8
BS = B_FULL // NCORE  # 128 batches per core
NCH = C // 128  # 4 chunks of 128 channels (PE contraction dim)
AC = 2 * HW + 1  # 99 augmented columns [Q | P | 1]
GRP = 16  # batches per DMA group
NGRP = BS // GRP
ITERS = 6
EPS_S = 0.05
TEMP = 12.5 / HW
EXP_BIAS = -4.0  # exp((sim-1)/eps) * e^16 rescale; cancels in us*K*vs

f32 = mybir.dt.float32
f16 = mybir.dt.float16
Alu = mybir.AluOpType
Act = mybir.ActivationFunctionType
AxX = mybir.AxisListType.X


def build_nc(debug=False):
    nc = bacc.Bacc(None, target_bir_lowering=False, debug=debug)
    JW = GRP * AC  # cols per chunk-slab in stage
    aug = nc.declare_dram_parameter(
        "aug", [NGRP, 128, NCH * JW], f16, isOutput=False
    )
    outp = nc.declare_dram_parameter("out", [BS, 1], f32, isOutput=True)

    FW = AC * AC  # 9801 flat row width

    with tile.TileContext(nc) as tc:
        with (
            tc.tile_pool(name="big", bufs=1) as big,
            tc.tile_pool(name="stage", bufs=4) as stg,
            tc.tile_pool(name="gcopy", bufs=8) as gcp,
            tc.tile_pool(name="work", bufs=3) as wrk,
            tc.tile_pool(name="small", bufs=1) as sml,
            tc.tile_pool(name="psum", bufs=8, space="PSUM") as pp,
        ):
            flatG = big.tile([BS, FW], f32, tag="flatG", name="flatG")

            # ---------------- Phase 1: DMA in + Gram + flatten ----------------
            NSPL = 8
            SW = NCH * JW // NSPL
            for g in range(NGRP):
                th = stg.tile([128, NCH * JW], f16, tag="h", name="hg")
                # loads live alone on the SP queue: a flatten on the same
                # FIFO queue would head-of-line block the next group's
                # prefetch behind compute
                for ss in range(NSPL):
                    nc.sync.dma_start(
                        th[:, ss * SW : (ss + 1) * SW],
                        aug[g, :, ss * SW : (ss + 1) * SW],
                    )
                for bb in range(GRP):
                    b = g * GRP + bb
                    ps = pp.tile([128, AC], f32, tag="gram", name="gram")
                    for j in range(NCH):
                        base = j * JW + bb * AC
                        # widen weights to 128 cols (spill into following slab
                        # data -> junk G rows 99..127, never read); the very
                        # last slab position must stay 99 wide
                        wid = AC if (bb == GRP - 1 and j == NCH - 1) else 128
                        nc.tensor.matmul(
                            ps[0:wid, :],
                            th[:, base : base + wid],
                            th[:, base : base + AC],
                            start=(j == 0),
                            stop=(j == NCH - 1),
                        )
                    gs = gcp.tile([AC, AC], f32, tag="gs", name="gs")
                    nc.vector.tensor_copy(gs[:], ps[0:AC, :])
                    # flatten [99, 99] -> one batch-major row; spread issue
                    # cost across gpsimd (SWDGE) + both HWDGE queues
                    dmae = (nc.gpsimd, nc.scalar)[b % 2]
                    dmae.dma_start(flatG[b : b + 1, :], gs[:])

            # ---------------- Phase 1.5: fixup to sim/K/marginals -------------
            G3 = flatG[:].rearrange("p (q c) -> p q c", c=AC)
            qtp = G3[:, 0:HW, HW : 2 * HW]  # [128, 49, 49] raw Q^T P
            ptq = G3[:, HW : 2 * HW, 0:HW]
            sq = flatG[:, (AC - 1) * AC : (AC - 1) * AC + HW]  # 1^T Q
            sp = flatG[:, (AC - 1) * AC + HW : (AC - 1) * AC + 2 * HW]  # 1^T P

            def dview(row0, col0):
                # [128, 49] diagonal view: (row0+m)*99 + col0+m, stride 100
                v = flatG[:, row0 * AC + col0 : row0 * AC + col0 + 1].copy()
                v.ap = mybir.VecI64Pair([list(v.ap[0])] + [[AC + 1, HW]])
                return v

            dq = dview(0, 0)  # diag(QtQ)
            dp = dview(HW, HW)  # diag(PtP)

            def s49(tag):
                return sml.tile([BS, HW], f32, tag=tag, name=tag)

            inq, inp_, t1, t2 = s49("inq"), s49("inp"), s49("t1"), s49("t2")
            aq, ap_ = s49("aq"), s49("ap")
            w1, w2, us, vs = s49("w1"), s49("w2"), s49("us"), s49("vs")
            kv, rkv = s49("kv"), s49("rkv")
            s2 = sml.tile([BS, 1], f32, tag="s2", name="s2")
            ebias = sml.tile([BS, 1], f32, tag="ebias", name="ebias")
            nc.vector.memset(ebias[:], EXP_BIAS)
            # warm the ACT sqrt/exp table sets early (no data deps -> Tile
            # schedules these under the phase-1 DMA shadow while ACT is idle,
            # hiding the ~2.7us-per-set PSEUDO_LOAD_ACT_FUNC_SET cost)
            wrm = sml.tile([BS, 1], f32, tag="wrm", name="wrm")
            nc.vector.memset(wrm[:], 1.0)
            nc.scalar.activation(wrm[:], wrm[:], Act.Sqrt)
            nc.scalar.activation(wrm[:], wrm[:], Act.Exp)
            lg = sml.tile([BS, 1], f32, tag="lg", name="lg")
            lgf = sml.tile([BS, 1], f32, tag="lgf", name="lgf")

            def v3(t):  # [128, 49, 49] view of a [128, 2401] tile
                return t[:].rearrange("p (q c) -> p q c", c=HW)

            def v3t(t):  # transposed view (strides 1, 49)
                return t[:].rearrange("p (q c) -> p c q", c=HW)

            # weight vectors: w = relu(rowsum/49) + 0.001 (unnormalized: the
            # r-normalization cancels in the logits, the c-normalization is a
            # final 1/s2 scale)
            nc.vector.tensor_reduce(w1[:], qtp, axis=AxX, op=Alu.add)
            nc.vector.tensor_reduce(w2[:], ptq, axis=AxX, op=Alu.add)
            for w in (w1, w2):
                nc.vector.tensor_scalar(w[:], w[:], 1.0 / HW, 0.0, Alu.mult, Alu.max)
                nc.vector.tensor_scalar(w[:], w[:], 0.001, None, Alu.add)
            nc.vector.tensor_reduce(s2[:], w2[:], axis=AxX, op=Alu.add)

            for (sx, dx, inv) in ((sq, dq, inq), (sp, dp, inp_)):
                # u = diag - s^2/C ; inv = rsqrt(u) via sqrt LUT+recip+Newton
                nc.vector.tensor_mul(t1[:], sx, sx)
                nc.vector.scalar_tensor_tensor(
                    t2[:], t1[:], -1.0 / C, dx, Alu.mult, Alu.add
                )
                nc.scalar.activation(t1[:], t2[:], Act.Sqrt)
                nc.vector.reciprocal(inv[:], t1[:])
                nc.vector.tensor_mul(t1[:], inv[:], inv[:])
                nc.vector.tensor_mul(t1[:], t1[:], t2[:])
                nc.vector.tensor_scalar(t1[:], t1[:], -0.5, 1.5, Alu.mult, Alu.add)
                nc.vector.tensor_mul(inv[:], inv[:], t1[:])

            rC = 1.0 / np.sqrt(float(C))
            nc.vector.scalar_tensor_tensor(
                aq[:], sq, rC, inq[:], Alu.mult, Alu.mult
            )
            nc.vector.scalar_tensor_tensor(
                ap_[:], sp, rC, inp_[:], Alu.mult, Alu.mult
            )

            simb = big.tile([BS, HW * HW], f32, tag="sim", name="sim")
            Kb = big.tile([BS, HW * HW], f32, tag="K", name="K")
            Ktb = big.tile([BS, HW * HW], f32, tag="Kt", name="Kt")
            b1 = wrk.tile([BS, HW * HW], f32, tag="w", name="b1")
            b3 = wrk.tile([BS, HW * HW], f32, tag="w", name="b3")
            simTb = wrk.tile([BS, HW * HW], f32, tag="w", name="simTb")

            bq = inq[:].unsqueeze(2).broadcast_to([BS, HW, HW])
            bp = inp_[:].unsqueeze(1).broadcast_to([BS, HW, HW])
            nc.vector.tensor_mul(v3(b1), bq, bp)  # B1 = inq x inp
            nc.vector.tensor_mul(v3(simb), qtp, v3(b1))  # B2
            baq = aq[:].unsqueeze(2).broadcast_to([BS, HW, HW])
            bap = ap_[:].unsqueeze(1).broadcast_to([BS, HW, HW])
            nc.vector.tensor_mul(v3(b3), baq, bap)  # B3 = aq x ap
            nc.vector.tensor_sub(v3(simb), v3(simb), v3(b3))  # sim = B2 - B3
            nc.vector.tensor_mul(v3(simTb), ptq, v3t(b1))
            nc.vector.tensor_sub(v3(simTb), v3(simTb), v3t(b3))
            nc.scalar.activation(
                Kb[:], simb[:], Act.Exp, scale=1.0 / EPS_S, bias=ebias[:]
            )
            nc.scalar.activation(
                Ktb[:], simTb[:], Act.Exp, scale=1.0 / EPS_S, bias=ebias[:]
            )

            # ---------------- Phase 2: Sinkhorn (Gauss-Seidel, linear) --------
            tb = wrk.tile([BS, HW * HW], f32, tag="w", name="tb")
            bvs = vs[:].unsqueeze(1).broadcast_to([BS, HW, HW])
            bus = us[:].unsqueeze(1).broadcast_to([BS, HW, HW])
            for it in range(ITERS):
                if it == 0:
                    nc.vector.tensor_reduce(kv[:], v3(Kb), axis=AxX, op=Alu.add)
                else:
                    nc.vector.tensor_mul(v3(tb), v3(Kb), bvs)
                    nc.vector.tensor_reduce(kv[:], v3(tb), axis=AxX, op=Alu.add)
                nc.vector.reciprocal(rkv[:], kv[:])
                nc.vector.tensor_mul(us[:], w1[:], rkv[:])
                nc.vector.tensor_mul(v3(tb), v3(Ktb), bus)
                nc.vector.tensor_reduce(kv[:], v3(tb), axis=AxX, op=Alu.add)
                nc.vector.reciprocal(rkv[:], kv[:])
                nc.vector.tensor_mul(vs[:], w2[:], rkv[:])

            # ---------------- Phase 3: logits ---------------------------------
            nc.vector.tensor_mul(v3(tb), v3(Kb), bvs)
            nc.vector.tensor_mul(tb[:], tb[:], simb[:])
            nc.vector.tensor_reduce(kv[:], v3(tb), axis=AxX, op=Alu.add)
            nc.vector.tensor_mul(kv[:], kv[:], us[:])
            nc.vector.tensor_reduce(lg[:], kv[:], axis=AxX, op=Alu.add)
            nc.vector.reciprocal(rkv[:, 0:1], s2[:])
            nc.vector.scalar_tensor_tensor(
                lgf[:], lg[:], TEMP, rkv[:, 0:1], Alu.mult, Alu.mult
            )  # (lg * T) / s2
            nc.sync.dma_start(outp[:, :], lgf[:])

    nc.compile()
    return nc


_NC = None


def _get_nc():
    global _NC
    if _NC is None:
        _NC = build_nc()
    return _NC


def _prep_in_maps(feature_map1, feature_map2):
    q = np.ascontiguousarray(np.asarray(feature_map1, dtype=np.float32)).reshape(
        B_FULL, C, HW
    )
    p = np.ascontiguousarray(np.asarray(feature_map2, dtype=np.float32)).reshape(
        B_FULL, C, HW
    )
    in_maps = []
    for i in range(NCORE):
        sl = slice(i * BS, (i + 1) * BS)
        a32 = np.empty((NCH, 128, BS, AC), np.float32)
        a32[..., AC - 1] = 1.0
        a32[..., 0:HW] = q[sl].reshape(BS, NCH, 128, HW).transpose(1, 2, 0, 3)
        a32[..., HW : 2 * HW] = p[sl].reshape(BS, NCH, 128, HW).transpose(1, 2, 0, 3)
        aug = a32.astype(np.float16)
        # sequence DRAM as [group, channel-partition, chunk, batch, col] so
        # group loads read contiguous spans
        aug = np.ascontiguousarray(
            aug.reshape(NCH, 128, NGRP, GRP, AC).transpose(2, 1, 0, 3, 4)
        ).reshape(NGRP, 128, NCH * GRP * AC)
        in_maps.append({"aug": aug})
    return in_maps


def run(feature_map1, feature_map2, trace=False):
    in_maps = _prep_in_maps(feature_map1, feature_map2)
    nc = _get_nc()
    res = run_bass_kernel_spmd(nc, in_maps, core_ids=list(range(NCORE)), trace=trace)
    out = np.concatenate(
        [np.asarray(res.results[i]["out"]).reshape(BS) for i in range(NCORE)]
    ).astype(np.float32)
    return out, res


def kernel(feature_map1, feature_map2):
    out, _ = run(feature_map1, feature_map2, trace=False)
    return out



# revision 4
# speedup vs baseline: 1.8047x; 1.8047x over previous
"""DeepEMD Trainium2 kernel: batched 49x49 entropic-OT (Sinkhorn) similarity.

Strategy (8 NeuronCores, data-parallel over batch):
- Host prepack does the cheap O(B*C*N) work exactly in fp32: channel-mean
  centering, unit-normalization of the node vectors, the weight vectors
  w1/w2 = relu(<A, mean(B)>)+1e-3 and the final T/sum(w2) scale. The device
  then only needs the cross Gram Q^T P (= the cosine similarity map), exp,
  and the Sinkhorn iterations.
- Each core gets 128 batches. Host packs, per (chunk j of 128 channels,
  batch b), [Qhat | Phat] (128 x 98) in fp16, grouped 16 batches per DMA
  span so loads are contiguous.
- PE computes sim_b = Qhat^T Phat per batch: 4 accumulating fp16 matmuls
  (weights = Qhat widened to 128 cols for fast-weight-load; junk rows
  49..127 ignored, moving = Phat 49 cols). 8 batches share one PSUM bank
  ([128, 392]); one vector copy per 8 batches moves rows 0..48 to SBUF.
- A per-batch SBUF->SBUF DMA flattens [49, 49] into row b of a [128, 2401]
  tile; everything after runs batch-on-partitions with full 128-lane DVE.
- Sinkhorn runs in the linear domain (K = exp((sim-1)/eps - 4)) with
  Gauss-Seidel updates us = w1/(K vs), vs = w2/(K^T us). The column-sum
  side uses a strided (transposed-view) reduce so K^T is never
  materialized. ITERS=4 gives ~3.5e-3 end-to-end (tolerance 2e-2).
- logits[b] = T/s2 * us^T ((K.sim) vs) via a fused tensor_tensor_reduce.
"""

import os
import sys

import numpy as np

sys.path.insert(0, "/opt/trn_rl_repo")

import concourse.bass as bass
import concourse.bacc as bacc
import concourse.mybir as mybir
from concourse import tile
from concourse.bass_utils import run_bass_kernel_spmd

B_FULL, C, HW = 1024, 512, 49
NCORE = 8
BS = B_FULL // NCORE  # 128 batches per core
NCH = C // 128  # 4 chunks of 128 channels (PE contraction dim)
W2 = 2 * HW  # 98 cols per (batch, chunk): [Qhat | Phat]
GRP = 16  # batches per DMA group
NGRP = BS // GRP
JW = GRP * W2  # 1568 cols per chunk-slab in a group
WCH = NCH * JW + 32  # +32 zero pad so the last 128-wide weight view is legal
FW = HW * HW  # 2401
ITERS = 4
EPS_S = 0.05
TEMP = 12.5 / HW
EXP_BIAS = -4.0  # K = exp((sim-1)/eps - 4); rescale cancels in us*K*vs

f32 = mybir.dt.float32
f16 = mybir.dt.float16
Alu = mybir.AluOpType
Act = mybir.ActivationFunctionType
AxX = mybir.AxisListType.X


def build_nc(debug=False):
    nc = bacc.Bacc(None, target_bir_lowering=False, debug=debug)
    aug = nc.declare_dram_parameter("aug", [NGRP, 128, WCH], f16, isOutput=False)
    auxp = nc.declare_dram_parameter("aux", [BS, HW + HW + 1], f32, isOutput=False)
    outp = nc.declare_dram_parameter("out", [BS, 1], f32, isOutput=True)

    with tile.TileContext(nc) as tc:
        with (
            tc.tile_pool(name="big", bufs=1) as big,
            tc.tile_pool(name="stage", bufs=3) as stg,
            tc.tile_pool(name="gcopy", bufs=4) as gcp,
            tc.tile_pool(name="work", bufs=2) as wrk,
            tc.tile_pool(name="small", bufs=1) as sml,
            tc.tile_pool(name="psum", bufs=4, space="PSUM") as pp,
        ):
            flatG = big.tile([BS, FW], f32, tag="flatG", name="flatG")
            aux = sml.tile([BS, HW + HW + 1], f32, tag="aux", name="aux")
            nc.sync.dma_start(aux[:], auxp[:, :])
            w1 = aux[:, 0:HW]
            w2 = aux[:, HW : 2 * HW]
            rs2t = aux[:, 2 * HW : 2 * HW + 1]

            ebias = sml.tile([BS, 1], f32, tag="ebias", name="ebias")
            nc.vector.memset(ebias[:], EXP_BIAS - 1.0 / EPS_S)
            # warm the ACT exp table set early (no data deps -> runs under the
            # phase-1 DMA shadow, hiding the ~2.7us PSEUDO_LOAD_ACT_FUNC_SET)
            wrm = sml.tile([BS, 1], f32, tag="wrm", name="wrm")
            nc.vector.memset(wrm[:], 1.0)
            nc.scalar.activation(wrm[:], wrm[:], Act.Exp)

            # ---------------- Phase 1: DMA in + cross-Gram + flatten ----------
            NSPL = 4
            SW = WCH // NSPL  # 1576
            for g in range(NGRP):
                th = stg.tile([128, WCH], f16, tag="h", name="hg")
                for ss in range(NSPL):
                    nc.sync.dma_start(
                        th[:, ss * SW : (ss + 1) * SW],
                        aug[g, :, ss * SW : (ss + 1) * SW],
                    )
                for half in range(2):
                    ps = pp.tile([128, 8 * HW], f32, tag="gram", name="gram")
                    for b8 in range(8):
                        bb = half * 8 + b8
                        for j in range(NCH):
                            woff = j * JW + bb * W2
                            nc.tensor.matmul(
                                ps[:, b8 * HW : (b8 + 1) * HW],
                                th[:, woff : woff + 128],
                                th[:, woff + HW : woff + W2],
                                start=(j == 0),
                                stop=(j == NCH - 1),
                            )
                    st = gcp.tile([HW, 8 * HW], f32, tag="gs", name="gs")
                    nc.vector.tensor_copy(st[:], ps[0:HW, :])
                    for b8 in range(8):
                        b = g * GRP + half * 8 + b8
                        dmae = (nc.gpsimd, nc.scalar, nc.sync)[b % 3]
                        dmae.dma_start(
                            flatG[b : b + 1, :], st[:, b8 * HW : (b8 + 1) * HW]
                        )

            # ---------------- Phase 2: K = exp, Sinkhorn ----------------------
            Kb = big.tile([BS, FW], f32, tag="K", name="K")
            tb = wrk.tile([BS, FW], f32, tag="w", name="tb")
            jk = wrk.tile([BS, FW], f32, tag="w", name="jk")

            def s49(tag):
                return sml.tile([BS, HW], f32, tag=tag, name=tag)

            us, vs, kv, rkv = s49("us"), s49("vs"), s49("kv"), s49("rkv")
            lg = sml.tile([BS, 1], f32, tag="lg", name="lg")
            lgf = sml.tile([BS, 1], f32, tag="lgf", name="lgf")

            def v3(t):  # [128, 49, 49] view of a [128, 2401] tile
                return t[:].rearrange("p (q c) -> p q c", c=HW)

            def v3t(t):  # transposed view: reduce X sums over q (stride 49)
                return t[:].rearrange("p (q c) -> p c q", c=HW)

            nc.scalar.activation(
                Kb[:], flatG[:], Act.Exp, scale=1.0 / EPS_S, bias=ebias[:]
            )

            bus = us[:].unsqueeze(2).broadcast_to([BS, HW, HW])
            bvs = vs[:].unsqueeze(1).broadcast_to([BS, HW, HW])
            for it in range(ITERS):
                if it == 0:
                    nc.vector.tensor_reduce(kv[:], v3(Kb), axis=AxX, op=Alu.add)
                else:
                    nc.vector.tensor_mul(v3(tb), v3(Kb), bvs)
                    nc.vector.tensor_reduce(kv[:], v3(tb), axis=AxX, op=Alu.add)
                nc.vector.reciprocal(rkv[:], kv[:])
                nc.vector.tensor_mul(us[:], w1, rkv[:])
                nc.vector.tensor_mul(v3(tb), v3(Kb), bus)
                nc.vector.tensor_reduce(kv[:], v3t(tb), axis=AxX, op=Alu.add)
                nc.vector.reciprocal(rkv[:], kv[:])
                nc.vector.tensor_mul(vs[:], w2, rkv[:])

            # ---------------- Phase 3: logits ---------------------------------
            nc.vector.tensor_mul(v3(tb), v3(Kb), bvs)
            nc.vector.tensor_mul(v3(tb), v3(tb), bus)
            nc.vector.tensor_mul(jk[:], tb[:], flatG[:])
            nc.vector.tensor_reduce(lg[:], jk[:], axis=AxX, op=Alu.add)
            nc.vector.tensor_mul(lgf[:], lg[:], rs2t)
            nc.sync.dma_start(outp[:, :], lgf[:])

    nc.compile()
    return nc


_NC = None


def _get_nc():
    global _NC
    if _NC is None:
        _NC = build_nc()
    return _NC


def _prep_in_maps(feature_map1, feature_map2):
    q = np.ascontiguousarray(np.asarray(feature_map1, dtype=np.float32)).reshape(
        B_FULL, C, HW
    )
    p = np.ascontiguousarray(np.asarray(feature_map2, dtype=np.float32)).reshape(
        B_FULL, C, HW
    )
    # weight vectors from the RAW features (exact, fp32)
    w1 = np.maximum((q * p.mean(axis=2, keepdims=True)).sum(axis=1), 0.0) + 0.001
    w2 = np.maximum((p * q.mean(axis=2, keepdims=True)).sum(axis=1), 0.0) + 0.001
    rs2t = (TEMP / w2.sum(axis=1, keepdims=True)).astype(np.float32)
    # center + unit-normalize the node vectors (exact, fp32), then fp16
    qc = q - q.mean(axis=1, keepdims=True)
    pc = p - p.mean(axis=1, keepdims=True)
    qn = (qc / (np.linalg.norm(qc, axis=1, keepdims=True) + 1e-8)).astype(np.float16)
    pn = (pc / (np.linalg.norm(pc, axis=1, keepdims=True) + 1e-8)).astype(np.float16)

    in_maps = []
    for i in range(NCORE):
        sl = slice(i * BS, (i + 1) * BS)
        # [NGRP, 128ch, NCH, GRP, 98] -> contiguous group spans
        a = np.zeros((NGRP, 128, WCH), np.float16)
        qi = qn[sl].reshape(NGRP, GRP, NCH, 128, HW).transpose(0, 3, 2, 1, 4)
        pi = pn[sl].reshape(NGRP, GRP, NCH, 128, HW).transpose(0, 3, 2, 1, 4)
        blk = np.empty((NGRP, 128, NCH, GRP, W2), np.float16)
        blk[..., 0:HW] = qi
        blk[..., HW:W2] = pi
        a[:, :, : NCH * JW] = blk.reshape(NGRP, 128, NCH * JW)
        aux = np.empty((BS, HW + HW + 1), np.float32)
        aux[:, 0:HW] = w1[sl]
        aux[:, HW : 2 * HW] = w2[sl]
        aux[:, 2 * HW :] = rs2t[sl]
        in_maps.append({"aug": a, "aux": aux})
    return in_maps


def run(feature_map1, feature_map2, trace=False):
    in_maps = _prep_in_maps(feature_map1, feature_map2)
    nc = _get_nc()
    res = run_bass_kernel_spmd(nc, in_maps, core_ids=list(range(NCORE)), trace=trace)
    out = np.concatenate(
        [np.asarray(res.results[i]["out"]).reshape(BS) for i in range(NCORE)]
    ).astype(np.float32)
    return out, res


def kernel(feature_map1, feature_map2):
    out, _ = run(feature_map1, feature_map2, trace=False)
    return out


# revision 7
# speedup vs baseline: 1.9990x; 1.1076x over previous
"""DeepEMD Trainium2 kernel: batched 49x49 entropic-OT (Sinkhorn) similarity.

Strategy (8 NeuronCores, data-parallel over batch):
- Host prepack does the cheap O(B*C*N) work exactly in fp32: channel-mean
  centering, unit-normalization of the node vectors, the weight vectors
  w1/w2 = relu(<A, mean(B)>)+1e-3 and the final T/sum(w2) scale. The device
  then only needs the cross Gram Q^T P (= the cosine similarity map), exp,
  and the Sinkhorn iterations.
- Each core gets 128 batches. Host packs, per (chunk j of 128 channels,
  batch b), [Qhat | Phat] (128 x 98) in fp16, grouped 16 batches per DMA
  span so loads are contiguous. Stage pool bufs=6 keeps the load stream
  ahead of the PE.
- PE computes sim_b = Qhat^T Phat per batch: 4 accumulating fp16 matmuls
  (weights = Qhat widened to 128 cols for fast-weight-load; junk rows
  49..127 ignored, moving = Phat 49 cols). 8 batches share one PSUM bank
  ([128, 392]); one vector copy per 8 batches moves rows 0..48 to SBUF.
- A per-batch SBUF->SBUF DMA flattens [49, 49] into row b of a [128, 49*50]
  tile (p padded to 50, pad = -5 so exp(pad) == 0); everything after runs
  batch-on-partitions with full 128-lane DVE, 16-bit where possible.
- Sinkhorn runs in the linear domain (K = exp((sim-1)/eps + 16), fp16) with
  Gauss-Seidel updates us = w1/(K vs), vs = w2/(K^T us). K^T is
  materialized by a second (strided-input) ACT exp so both reduce
  directions are contiguous. ITERS=4 gives ~3.5e-3 (tolerance 2e-2).
- logits[b] = T/s2 * us^T ((K.sim) vs) via a fused affine_mul_reduce.
"""

import os
import sys

import numpy as np

sys.path.insert(0, "/opt/trn_rl_repo")

import concourse.bass as bass
import concourse.bacc as bacc
import concourse.mybir as mybir
from concourse import tile
from concourse.bass_utils import run_bass_kernel_spmd

B_FULL, C, HW = 1024, 512, 49
HP = HW + 1  # padded inner dim 50
NCORE = 8
BS = B_FULL // NCORE  # 128 batches per core
NCH = C // 128  # 4 chunks of 128 channels (PE contraction dim)
W2 = 2 * HW  # 98 cols per (batch, chunk): [Qhat | Phat]
GRP = 16  # batches per DMA group
NGRP = BS // GRP
JW = GRP * W2  # 1568 cols per chunk-slab in a group
WCH = NCH * JW + 32  # +32 zero pad so the last 128-wide weight view is legal
FWP = HW * HP  # 2450 padded flat row
ITERS = 4
EPS_S = 0.05
TEMP = 12.5 / HW
EXP_BIAS = 16.0  # K = exp((sim-1)/eps + 16): fp16-safe range [2e-4, 1.35]
PAD = -5.0  # sim pad value; exp((PAD-1)/eps+16) == 0 in fp16

f32 = mybir.dt.float32
f16 = mybir.dt.float16
Alu = mybir.AluOpType
Act = mybir.ActivationFunctionType
AxX = mybir.AxisListType.X


def build_nc(debug=False):
    nc = bacc.Bacc(None, target_bir_lowering=False, debug=debug)
    aug = nc.declare_dram_parameter("aug", [NGRP, 128, WCH], f16, isOutput=False)
    auxp = nc.declare_dram_parameter("aux", [BS, HW + HW + 1], f32, isOutput=False)
    outp = nc.declare_dram_parameter("out", [BS, 1], f32, isOutput=True)

    with tile.TileContext(nc) as tc:
        with (
            tc.tile_pool(name="big", bufs=1) as big,
            tc.tile_pool(name="stage", bufs=6) as stg,
            tc.tile_pool(name="gcopy", bufs=4) as gcp,
            tc.tile_pool(name="small", bufs=1) as sml,
            tc.tile_pool(name="psum", bufs=4, space="PSUM") as pp,
        ):
            flatG = big.tile([BS, FWP], f32, tag="flatG", name="flatG")
            nc.vector.memset(flatG[:], PAD)
            aux = sml.tile([BS, HW + HW + 1], f32, tag="aux", name="aux")
            nc.sync.dma_start(aux[:], auxp[:, :])
            w1 = aux[:, 0:HW]
            w2 = aux[:, HW : 2 * HW]
            rs2t = aux[:, 2 * HW : 2 * HW + 1]

            ebias = sml.tile([BS, 1], f32, tag="ebias", name="ebias")
            nc.vector.memset(ebias[:], EXP_BIAS - 1.0 / EPS_S)
            # warm the ACT exp table set early (no data deps -> runs under the
            # phase-1 DMA shadow, hiding the ~2.7us PSEUDO_LOAD_ACT_FUNC_SET)
            wrm = sml.tile([BS, 1], f32, tag="wrm", name="wrm")
            nc.vector.memset(wrm[:], 1.0)
            nc.scalar.activation(wrm[:], wrm[:], Act.Exp)

            # ---------------- Phase 1: DMA in + cross-Gram + flatten ----------
            NSPL = 4
            SW = WCH // NSPL  # 1576
            for g in range(NGRP):
                th = stg.tile([128, WCH], f16, tag="h", name="hg")
                for ss in range(NSPL):
                    nc.sync.dma_start(
                        th[:, ss * SW : (ss + 1) * SW],
                        aug[g, :, ss * SW : (ss + 1) * SW],
                    )
                for half in range(2):
                    ps = pp.tile([128, 8 * HW], f32, tag="gram", name="gram")
                    for b8 in range(8):
                        bb = half * 8 + b8
                        for j in range(NCH):
                            woff = j * JW + bb * W2
                            nc.tensor.matmul(
                                ps[:, b8 * HW : (b8 + 1) * HW],
                                th[:, woff : woff + 128],
                                th[:, woff + HW : woff + W2],
                                start=(j == 0),
                                stop=(j == NCH - 1),
                            )
                    st = gcp.tile([HW, 8 * HW], f32, tag="gs", name="gs")
                    nc.vector.tensor_copy(st[:], ps[0:HW, :])
                    for b8 in range(8):
                        b = g * GRP + half * 8 + b8
                        dmae = (nc.gpsimd, nc.scalar, nc.sync)[b % 3]
                        dst = flatG[b : b + 1, :].rearrange(
                            "b (q c) -> b q c", c=HP
                        )[:, :, 0:HW]
                        dmae.dma_start(dst, st[:, b8 * HW : (b8 + 1) * HW])

            # ---------------- Phase 2: K = exp, Sinkhorn (fp16) ---------------
            Kb = big.tile([BS, FWP], f16, tag="K", name="K")  # [b, q, p50]
            Kt = big.tile([BS, FWP], f16, tag="Kt", name="Kt")  # [b, p, q50]
            nc.vector.memset(Kt[:], 0.0)
            tb = big.tile([BS, FWP], f16, tag="tb", name="tb")
            tb2 = big.tile([BS, FWP], f16, tag="tb2", name="tb2")
            jk = big.tile([BS, FWP], f32, tag="jk", name="jk")

            us = sml.tile([BS, HP], f16, tag="us", name="us")
            vs = sml.tile([BS, HP], f16, tag="vs", name="vs")
            nc.vector.memset(us[:], 0.0)
            nc.vector.memset(vs[:], 0.0)
            kv = sml.tile([BS, HW], f32, tag="kv", name="kv")
            rkv = sml.tile([BS, HW], f32, tag="rkv", name="rkv")
            lg = sml.tile([BS, 1], f32, tag="lg", name="lg")
            lgf = sml.tile([BS, 1], f32, tag="lgf", name="lgf")

            def v3(t):  # [128, 49, 50] view
                return t[:].rearrange("p (q c) -> p q c", c=HP)

            # K = exp((sim-1)/eps + 16); pad cols exp(-104) flush to 0 in fp16
            nc.scalar.activation(
                Kb[:], flatG[:], Act.Exp, scale=1.0 / EPS_S, bias=ebias[:]
            )
            # K^T via strided-input exp: in [b, p, q] (inner stride 50),
            # out [b, p, q] contiguous rows of 49 (pad q col stays 0)
            simT = flatG[:].rearrange("b (q c) -> b c q", c=HP)[:, 0:HW, :]
            KtV = v3(Kt)[:, :, 0:HW]
            nc.scalar.activation(KtV, simT, Act.Exp, scale=1.0 / EPS_S, bias=ebias[:])

            bus = us[:].unsqueeze(1).broadcast_to([BS, HW, HP])
            bvs = vs[:].unsqueeze(1).broadcast_to([BS, HW, HP])
            for it in range(ITERS):
                if it == 0:
                    nc.vector.tensor_reduce(kv[:], v3(Kb), axis=AxX, op=Alu.add)
                else:
                    nc.vector.tensor_mul(v3(tb), v3(Kb), bvs)
                    nc.vector.tensor_reduce(kv[:], v3(tb), axis=AxX, op=Alu.add)
                nc.vector.reciprocal(rkv[:], kv[:])
                nc.vector.tensor_mul(us[:, 0:HW], w1, rkv[:])
                nc.vector.tensor_mul(v3(tb2), v3(Kt), bus)
                nc.vector.tensor_reduce(kv[:], v3(tb2), axis=AxX, op=Alu.add)
                nc.vector.reciprocal(rkv[:], kv[:])
                nc.vector.tensor_mul(vs[:, 0:HW], w2, rkv[:])

            # ---------------- Phase 3: logits ---------------------------------
            nc.vector.tensor_mul(v3(tb), v3(Kb), bvs)
            bqs = us[:, 0:HW].unsqueeze(2).broadcast_to([BS, HW, HP])
            nc.vector.tensor_mul(v3(tb2), v3(tb), bqs)
            nc.vector.tensor_mul(jk[:], tb2[:], flatG[:])
            nc.vector.tensor_reduce(lg[:], jk[:], axis=AxX, op=Alu.add)
            nc.vector.tensor_mul(lgf[:], lg[:], rs2t)
            nc.sync.dma_start(outp[:, :], lgf[:])

    nc.compile()
    return nc


_NC = None


def _get_nc():
    global _NC
    if _NC is None:
        _NC = build_nc()
    return _NC


def _prep_in_maps(feature_map1, feature_map2):
    q = np.ascontiguousarray(np.asarray(feature_map1, dtype=np.float32)).reshape(
        B_FULL, C, HW
    )
    p = np.ascontiguousarray(np.asarray(feature_map2, dtype=np.float32)).reshape(
        B_FULL, C, HW
    )
    # weight vectors from the RAW features (exact, fp32)
    w1 = np.maximum((q * p.mean(axis=2, keepdims=True)).sum(axis=1), 0.0) + 0.001
    w2 = np.maximum((p * q.mean(axis=2, keepdims=True)).sum(axis=1), 0.0) + 0.001
    rs2t = (TEMP / w2.sum(axis=1, keepdims=True)).astype(np.float32)
    # center + unit-normalize the node vectors (exact, fp32), then fp16
    qc = q - q.mean(axis=1, keepdims=True)
    pc = p - p.mean(axis=1, keepdims=True)
    qn = (qc / (np.linalg.norm(qc, axis=1, keepdims=True) + 1e-8)).astype(np.float16)
    pn = (pc / (np.linalg.norm(pc, axis=1, keepdims=True) + 1e-8)).astype(np.float16)

    in_maps = []
    for i in range(NCORE):
        sl = slice(i * BS, (i + 1) * BS)
        # [NGRP, 128ch, NCH, GRP, 98] -> contiguous group spans
        a = np.zeros((NGRP, 128, WCH), np.float16)
        qi = qn[sl].reshape(NGRP, GRP, NCH, 128, HW).transpose(0, 3, 2, 1, 4)
        pi = pn[sl].reshape(NGRP, GRP, NCH, 128, HW).transpose(0, 3, 2, 1, 4)
        blk = np.empty((NGRP, 128, NCH, GRP, W2), np.float16)
        blk[..., 0:HW] = qi
        blk[..., HW:W2] = pi
        a[:, :, : NCH * JW] = blk.reshape(NGRP, 128, NCH * JW)
        aux = np.empty((BS, HW + HW + 1), np.float32)
        aux[:, 0:HW] = w1[sl]
        aux[:, HW : 2 * HW] = w2[sl]
        aux[:, 2 * HW :] = rs2t[sl]
        in_maps.append({"aug": a, "aux": aux})
    return in_maps


def run(feature_map1, feature_map2, trace=False):
    in_maps = _prep_in_maps(feature_map1, feature_map2)
    nc = _get_nc()
    res = run_bass_kernel_spmd(nc, in_maps, core_ids=list(range(NCORE)), trace=trace)
    out = np.concatenate(
        [np.asarray(res.results[i]["out"]).reshape(BS) for i in range(NCORE)]
    ).astype(np.float32)
    return out, res


def kernel(feature_map1, feature_map2):
    out, _ = run(feature_map1, feature_map2, trace=False)
    return out


# revision 8
# speedup vs baseline: 2.0691x; 1.0351x over previous
"""DeepEMD Trainium2 kernel: batched 49x49 entropic-OT (Sinkhorn) similarity.

Strategy (8 NeuronCores, data-parallel over batch):
- Host prepack does the cheap O(B*C*N) work exactly in fp32: channel-mean
  centering, unit-normalization of the node vectors, the weight vectors
  w1/w2 = relu(<A, mean(B)>)+1e-3 and the final T/sum(w2) scale. The device
  then only needs the cross Gram Q^T P (= the cosine similarity map), exp,
  and the Sinkhorn iterations.
- Each core gets 128 batches. Host packs, per (chunk j of 128 channels,
  batch b), [Qhat | Phat] (128 x 98) in fp16, grouped 16 batches per DMA
  span so loads are contiguous. Stage pool bufs=6 keeps the load stream
  ahead of the PE.
- PE computes sim_b = Qhat^T Phat per batch: 4 accumulating fp16 matmuls
  (weights = Qhat widened to 128 cols for fast-weight-load; junk rows
  49..127 ignored, moving = Phat 49 cols). 8 batches share one PSUM bank
  ([128, 392]); one vector copy per 8 batches moves rows 0..48 to SBUF.
- A per-batch SBUF->SBUF DMA flattens [49, 49] into row b of a [128, 49*50]
  tile (p padded to 50, pad = -5 so exp(pad) == 0); everything after runs
  batch-on-partitions with full 128-lane DVE, 16-bit where possible.
- Sinkhorn runs in the linear domain (K = exp((sim-1)/eps + 16), fp16) with
  Gauss-Seidel updates us = w1/(K vs), vs = w2/(K^T us). K^T is
  materialized by a second (strided-input) ACT exp so both reduce
  directions are contiguous. ITERS=4 gives ~3.5e-3 (tolerance 2e-2).
- logits[b] = T/s2 * us^T ((K.sim) vs) via a fused affine_mul_reduce.
"""

import os
import sys

import numpy as np

sys.path.insert(0, "/opt/trn_rl_repo")

import concourse.bass as bass
import concourse.bacc as bacc
import concourse.mybir as mybir
from concourse import tile
from concourse.bass_utils import run_bass_kernel_spmd

B_FULL, C, HW = 1024, 512, 49
HP = HW + 1  # padded inner dim 50
NCORE = 8
BS = B_FULL // NCORE  # 128 batches per core
NCH = C // 128  # 4 chunks of 128 channels (PE contraction dim)
W2 = 2 * HW  # 98 cols per (batch, chunk): [Qhat | Phat]
GRP = 16  # batches per DMA group
NGRP = BS // GRP
JW = GRP * W2  # 1568 cols per chunk-slab in a group
WCH = NCH * JW + 32  # +32 zero pad so the last 128-wide weight view is legal
FWP = HW * HP  # 2450 padded flat row
ITERS = 4
EPS_S = 0.05
TEMP = 12.5 / HW
EXP_BIAS = 16.0  # K = exp((sim-1)/eps + 16): fp16-safe range [2e-4, 1.35]
PAD = -5.0  # sim pad value; exp((PAD-1)/eps+16) == 0 in fp16

f32 = mybir.dt.float32
f16 = mybir.dt.float16
Alu = mybir.AluOpType
Act = mybir.ActivationFunctionType
AxX = mybir.AxisListType.X


def build_nc(debug=False):
    nc = bacc.Bacc(None, target_bir_lowering=False, debug=debug)
    aug = nc.declare_dram_parameter("aug", [NGRP, 128, WCH], f16, isOutput=False)
    auxp = nc.declare_dram_parameter("aux", [BS, HW + HW + 1], f32, isOutput=False)
    outp = nc.declare_dram_parameter("out", [BS, 1], f32, isOutput=True)

    with tile.TileContext(nc) as tc:
        with (
            tc.tile_pool(name="big", bufs=1) as big,
            tc.tile_pool(name="stage", bufs=6) as stg,
            tc.tile_pool(name="gcopy", bufs=4) as gcp,
            tc.tile_pool(name="small", bufs=1) as sml,
            tc.tile_pool(name="psum", bufs=4, space="PSUM") as pp,
        ):
            flatG = big.tile([BS, FWP], f32, tag="flatG", name="flatG")
            nc.vector.memset(flatG[:], PAD)
            aux = sml.tile([BS, HW + HW + 1], f32, tag="aux", name="aux")
            nc.sync.dma_start(aux[:], auxp[:, :])
            w1 = aux[:, 0:HW]
            w2 = aux[:, HW : 2 * HW]
            rs2t = aux[:, 2 * HW : 2 * HW + 1]

            ebias = sml.tile([BS, 1], f32, tag="ebias", name="ebias")
            nc.vector.memset(ebias[:], EXP_BIAS - 1.0 / EPS_S)
            # warm the ACT exp table set early (no data deps -> runs under the
            # phase-1 DMA shadow, hiding the ~2.7us PSEUDO_LOAD_ACT_FUNC_SET)
            wrm = sml.tile([BS, 1], f32, tag="wrm", name="wrm")
            nc.vector.memset(wrm[:], 1.0)
            nc.scalar.activation(wrm[:], wrm[:], Act.Exp)

            # ---------------- Phase 1: DMA in + cross-Gram + flatten ----------
            NSPL = 4
            SW = WCH // NSPL  # 1576
            for g in range(NGRP):
                th = stg.tile([128, WCH], f16, tag="h", name="hg")
                for ss in range(NSPL):
                    nc.sync.dma_start(
                        th[:, ss * SW : (ss + 1) * SW],
                        aug[g, :, ss * SW : (ss + 1) * SW],
                    )
                for half in range(2):
                    ps = pp.tile([128, 8 * HW], f32, tag="gram", name="gram")
                    for b8 in range(8):
                        bb = half * 8 + b8
                        for j in range(NCH):
                            woff = j * JW + bb * W2
                            nc.tensor.matmul(
                                ps[:, b8 * HW : (b8 + 1) * HW],
                                th[:, woff : woff + 128],
                                th[:, woff + HW : woff + W2],
                                start=(j == 0),
                                stop=(j == NCH - 1),
                            )
                    st = gcp.tile([HW, 8 * HW], f32, tag="gs", name="gs")
                    nc.vector.tensor_copy(st[:], ps[0:HW, :])
                    for b8 in range(8):
                        b = g * GRP + half * 8 + b8
                        dmae = (nc.gpsimd, nc.scalar, nc.sync)[b % 3]
                        dst = flatG[b : b + 1, :].rearrange(
                            "b (q c) -> b q c", c=HP
                        )[:, :, 0:HW]
                        dmae.dma_start(dst, st[:, b8 * HW : (b8 + 1) * HW])

            # ---------------- Phase 2: K = exp, Sinkhorn (fp16) ---------------
            Kb = big.tile([BS, FWP], f16, tag="K", name="K")  # [b, q, p50]
            Kt = big.tile([BS, FWP], f16, tag="Kt", name="Kt")  # [b, p, q50]
            nc.vector.memset(Kt[:], 0.0)
            tb = big.tile([BS, FWP], f16, tag="tb", name="tb")
            tb2 = big.tile([BS, FWP], f16, tag="tb2", name="tb2")
            jk = big.tile([BS, FWP], f32, tag="jk", name="jk")

            us = sml.tile([BS, HP], f16, tag="us", name="us")
            vs = sml.tile([BS, HP], f16, tag="vs", name="vs")
            nc.vector.memset(us[:], 0.0)
            nc.vector.memset(vs[:], 0.0)
            kv = sml.tile([BS, HW], f32, tag="kv", name="kv")
            rkv = sml.tile([BS, HW], f32, tag="rkv", name="rkv")
            lg = sml.tile([BS, 1], f32, tag="lg", name="lg")
            lgf = sml.tile([BS, 1], f32, tag="lgf", name="lgf")

            def v3(t):  # [128, 49, 50] view
                return t[:].rearrange("p (q c) -> p q c", c=HP)

            # K = exp((sim-1)/eps + 16); pad cols exp(-104) flush to 0 in fp16
            nc.scalar.activation(
                Kb[:], flatG[:], Act.Exp, scale=1.0 / EPS_S, bias=ebias[:]
            )
            # K^T via strided-input exp: in [b, p, q] (inner stride 50),
            # out [b, p, q] contiguous rows of 49 (pad q col stays 0)
            simT = flatG[:].rearrange("b (q c) -> b c q", c=HP)[:, 0:HW, :]
            KtV = v3(Kt)[:, :, 0:HW]
            nc.scalar.activation(KtV, simT, Act.Exp, scale=1.0 / EPS_S, bias=ebias[:])

            bus = us[:].unsqueeze(1).broadcast_to([BS, HW, HP])
            bvs = vs[:].unsqueeze(1).broadcast_to([BS, HW, HP])
            for it in range(ITERS):
                if it == 0:
                    nc.vector.tensor_reduce(kv[:], v3(Kb), axis=AxX, op=Alu.add)
                else:
                    nc.vector.tensor_mul(v3(tb), v3(Kb), bvs)
                    nc.vector.tensor_reduce(kv[:], v3(tb), axis=AxX, op=Alu.add)
                nc.vector.reciprocal(rkv[:], kv[:])
                nc.vector.tensor_mul(us[:, 0:HW], w1, rkv[:])
                nc.vector.tensor_mul(v3(tb2), v3(Kt), bus)
                nc.vector.tensor_reduce(kv[:], v3(tb2), axis=AxX, op=Alu.add)
                nc.vector.reciprocal(rkv[:], kv[:])
                nc.vector.tensor_mul(vs[:, 0:HW], w2, rkv[:])

            # ---------------- Phase 3: logits ---------------------------------
            nc.vector.tensor_mul(v3(tb), v3(Kb), bvs)
            bqs = us[:, 0:HW].unsqueeze(2).broadcast_to([BS, HW, HP])
            nc.vector.tensor_mul(v3(tb2), v3(tb), bqs)
            nc.vector.affine_mul_reduce(
                out=jk[:],
                accum_out=lg[:],
                in0=tb2[:],
                in1=flatG[:],
                scale=1.0,
                bias=0.0,
            )
            nc.vector.tensor_mul(lgf[:], lg[:], rs2t)
            nc.sync.dma_start(outp[:, :], lgf[:])

    nc.compile()
    return nc


_NC = None


def _get_nc():
    global _NC
    if _NC is None:
        _NC = build_nc()
    return _NC


def _prep_in_maps(feature_map1, feature_map2):
    q = np.ascontiguousarray(np.asarray(feature_map1, dtype=np.float32)).reshape(
        B_FULL, C, HW
    )
    p = np.ascontiguousarray(np.asarray(feature_map2, dtype=np.float32)).reshape(
        B_FULL, C, HW
    )
    # weight vectors from the RAW features (exact, fp32)
    w1 = np.maximum((q * p.mean(axis=2, keepdims=True)).sum(axis=1), 0.0) + 0.001
    w2 = np.maximum((p * q.mean(axis=2, keepdims=True)).sum(axis=1), 0.0) + 0.001
    rs2t = (TEMP / w2.sum(axis=1, keepdims=True)).astype(np.float32)
    # center + unit-normalize the node vectors (exact, fp32), then fp16
    qc = q - q.mean(axis=1, keepdims=True)
    pc = p - p.mean(axis=1, keepdims=True)
    qn = (qc / (np.linalg.norm(qc, axis=1, keepdims=True) + 1e-8)).astype(np.float16)
    pn = (pc / (np.linalg.norm(pc, axis=1, keepdims=True) + 1e-8)).astype(np.float16)

    in_maps = []
    for i in range(NCORE):
        sl = slice(i * BS, (i + 1) * BS)
        # [NGRP, 128ch, NCH, GRP, 98] -> contiguous group spans
        a = np.zeros((NGRP, 128, WCH), np.float16)
        qi = qn[sl].reshape(NGRP, GRP, NCH, 128, HW).transpose(0, 3, 2, 1, 4)
        pi = pn[sl].reshape(NGRP, GRP, NCH, 128, HW).transpose(0, 3, 2, 1, 4)
        blk = np.empty((NGRP, 128, NCH, GRP, W2), np.float16)
        blk[..., 0:HW] = qi
        blk[..., HW:W2] = pi
        a[:, :, : NCH * JW] = blk.reshape(NGRP, 128, NCH * JW)
        aux = np.empty((BS, HW + HW + 1), np.float32)
        aux[:, 0:HW] = w1[sl]
        aux[:, HW : 2 * HW] = w2[sl]
        aux[:, 2 * HW :] = rs2t[sl]
        in_maps.append({"aug": a, "aux": aux})
    return in_maps


def run(feature_map1, feature_map2, trace=False):
    in_maps = _prep_in_maps(feature_map1, feature_map2)
    nc = _get_nc()
    res = run_bass_kernel_spmd(nc, in_maps, core_ids=list(range(NCORE)), trace=trace)
    out = np.concatenate(
        [np.asarray(res.results[i]["out"]).reshape(BS) for i in range(NCORE)]
    ).astype(np.float32)
    return out, res


def kernel(feature_map1, feature_map2):
    out, _ = run(feature_map1, feature_map2, trace=False)
    return out


# revision 13
# speedup vs baseline: 2.2185x; 1.0722x over previous
"""DeepEMD Trainium2 kernel: batched 49x49 entropic-OT (Sinkhorn) similarity.

Strategy (8 NeuronCores, data-parallel over batch):
- Host prepack does the cheap O(B*C*N) work exactly in fp32: channel-mean
  centering, unit-normalization of the node vectors, the weight vectors
  w1/w2 = relu(<A, mean(B)>)+1e-3 and the final T/sum(w2) scale. The device
  then only needs the cross Gram Q^T P (= the cosine similarity map), exp,
  and the Sinkhorn iterations.
- Each core gets 128 batches. Host packs, per (chunk j of 128 channels,
  batch b), [Qhat | Phat] (128 x 98) in fp16, grouped 16 batches per DMA
  span so loads are contiguous. Stage pool bufs=6 keeps the load stream
  ahead of the PE.
- PE computes sim_b = Qhat^T Phat per batch: 4 accumulating fp16 matmuls
  (weights = Qhat widened to 128 cols for fast-weight-load; junk rows
  49..127 ignored, moving = Phat 49 cols). 8 batches share one PSUM bank
  ([128, 392]); one vector copy per 8 batches moves rows 0..48 to SBUF.
- A per-batch SBUF->SBUF DMA flattens [49, 49] into row b of a [128, 49*50]
  tile (p padded to 50, pad = -5 so exp(pad) == 0); everything after runs
  batch-on-partitions with full 128-lane DVE, 16-bit where possible.
- Sinkhorn runs in the linear domain (K = exp((sim-1)/eps + 16), fp16) with
  Gauss-Seidel updates us = w1/(K vs), vs = w2/(K^T us). K^T is
  materialized by a second (strided-input) ACT exp so both reduce
  directions are contiguous. ITERS=4 gives ~3.5e-3 (tolerance 2e-2).
- logits[b] = T/s2 * us^T ((K.sim) vs) via a fused affine_mul_reduce.
"""

import os
import sys

import numpy as np

sys.path.insert(0, "/opt/trn_rl_repo")

import concourse.bass as bass
import concourse.bacc as bacc
import concourse.mybir as mybir
from concourse import tile
from concourse.bass_utils import run_bass_kernel_spmd

B_FULL, C, HW = 1024, 512, 49
HP = HW + 1  # padded inner dim 50
NCORE = 8
BS = B_FULL // NCORE  # 128 batches per core
NCH = C // 128  # 4 chunks of 128 channels (PE contraction dim)
W2 = 2 * HW  # 98 cols per (batch, chunk): [Qhat | Phat]
GRP = 16  # batches per DMA group
NGRP = BS // GRP
JW = GRP * W2  # 1568 cols per chunk-slab in a group
WCH = NCH * JW + 32  # +32 zero pad so the last 128-wide weight view is legal
FWP = HW * HP  # 2450 padded flat row
ITERS = 4
EPS_S = 0.05
TEMP = 12.5 / HW
EXP_BIAS = 16.0  # K = exp((sim-1)/eps + 16): fp16-safe range [2e-4, 1.35]
PAD = -5.0  # sim pad value; exp((PAD-1)/eps+16) == 0 in fp16

f32 = mybir.dt.float32
f16 = mybir.dt.float16
Alu = mybir.AluOpType
Act = mybir.ActivationFunctionType
AxX = mybir.AxisListType.X


def build_nc(debug=False):
    nc = bacc.Bacc(None, target_bir_lowering=False, debug=debug)
    aug = nc.declare_dram_parameter("aug", [NGRP, 128, WCH], f16, isOutput=False)
    auxp = nc.declare_dram_parameter("aux", [BS, HW + HW + 1], f32, isOutput=False)
    outp = nc.declare_dram_parameter("out", [BS, 1], f32, isOutput=True)

    with tile.TileContext(nc) as tc:
        with (
            tc.tile_pool(name="big", bufs=1) as big,
            tc.tile_pool(name="stage", bufs=6) as stg,
            tc.tile_pool(name="gcopy", bufs=4) as gcp,
            tc.tile_pool(name="small", bufs=1) as sml,
            tc.tile_pool(name="psum", bufs=4, space="PSUM") as pp,
        ):
            flatG = big.tile([BS, FWP], f32, tag="flatG", name="flatG")
            nc.vector.memset(flatG[:], PAD)
            aux = sml.tile([BS, HW + HW + 1], f32, tag="aux", name="aux")
            nc.sync.dma_start(aux[:], auxp[:, :])
            w1 = aux[:, 0:HW]
            w2 = aux[:, HW : 2 * HW]
            rs2t = aux[:, 2 * HW : 2 * HW + 1]

            ebias = sml.tile([BS, 1], f32, tag="ebias", name="ebias")
            nc.vector.memset(ebias[:], EXP_BIAS - 1.0 / EPS_S)
            # warm the ACT exp table set early (no data deps -> runs under the
            # phase-1 DMA shadow, hiding the ~2.7us PSEUDO_LOAD_ACT_FUNC_SET)
            wrm = sml.tile([BS, 1], f32, tag="wrm", name="wrm")
            nc.vector.memset(wrm[:], 1.0)
            nc.scalar.activation(wrm[:], wrm[:], Act.Exp)

            # ---------------- Phase 1: DMA in + cross-Gram + flatten ----------
            NSPL = 4
            SW = WCH // NSPL  # 1576
            for g in range(NGRP):
                th = stg.tile([128, WCH], f16, tag="h", name="hg")
                for ss in range(NSPL):
                    nc.sync.dma_start(
                        th[:, ss * SW : (ss + 1) * SW],
                        aug[g, :, ss * SW : (ss + 1) * SW],
                    )
                for half in range(2):
                    ps = pp.tile([128, 8 * HW], f32, tag="gram", name="gram")
                    for b8 in range(8):
                        bb = half * 8 + b8
                        for j in range(NCH):
                            woff = j * JW + bb * W2
                            nc.tensor.matmul(
                                ps[:, b8 * HW : (b8 + 1) * HW],
                                th[:, woff : woff + 128],
                                th[:, woff + HW : woff + W2],
                                start=(j == 0),
                                stop=(j == NCH - 1),
                            )
                    st = gcp.tile([HW, 8 * HW], f32, tag="gs", name="gs")
                    nc.vector.tensor_copy(st[:], ps[0:HW, :])
                    for b8 in range(8):
                        b = g * GRP + half * 8 + b8
                        dmae = (nc.gpsimd, nc.scalar)[b % 2]
                        dst = flatG[b : b + 1, :].rearrange(
                            "b (q c) -> b q c", c=HP
                        )[:, :, 0:HW]
                        dmae.dma_start(dst, st[:, b8 * HW : (b8 + 1) * HW])

            # ---------------- Phase 2: K = exp, Sinkhorn (fp16) ---------------
            Kb = big.tile([BS, FWP], f16, tag="K", name="K")  # [b, q, p50]
            Kt = big.tile([BS, FWP], f16, tag="Kt", name="Kt")  # [b, p, q50]
            nc.vector.memset(Kt[:], 0.0)
            tb = big.tile([BS, FWP], f16, tag="tb", name="tb")
            tb2 = big.tile([BS, FWP], f16, tag="tb2", name="tb2")
            jk = big.tile([BS, FWP], f32, tag="jk", name="jk")

            us = sml.tile([BS, HP], f16, tag="us", name="us")
            vs = sml.tile([BS, HP], f16, tag="vs", name="vs")
            nc.vector.memset(us[:], 0.0)
            nc.vector.memset(vs[:], 0.0)
            kv = sml.tile([BS, HW], f32, tag="kv", name="kv")
            rkv = sml.tile([BS, HW], f32, tag="rkv", name="rkv")
            lg = sml.tile([BS, 1], f32, tag="lg", name="lg")
            lgf = sml.tile([BS, 1], f32, tag="lgf", name="lgf")

            def v3(t):  # [128, 49, 50] view
                return t[:].rearrange("p (q c) -> p q c", c=HP)

            # K = exp((sim-1)/eps + 16); pad cols exp(-104) flush to 0 in fp16
            nc.scalar.activation(
                Kb[:], flatG[:], Act.Exp, scale=1.0 / EPS_S, bias=ebias[:]
            )
            # K^T via strided-input exp: in [b, p, q] (inner stride 50),
            # out [b, p, q] contiguous rows of 49 (pad q col stays 0)
            simT = flatG[:].rearrange("b (q c) -> b c q", c=HP)[:, 0:HW, :]
            KtV = v3(Kt)[:, :, 0:HW]
            nc.scalar.activation(KtV, simT, Act.Exp, scale=1.0 / EPS_S, bias=ebias[:])

            bus = us[:].unsqueeze(1).broadcast_to([BS, HW, HP])
            bvs = vs[:].unsqueeze(1).broadcast_to([BS, HW, HP])
            # ends on the u-update: flow row sums = w1 exactly, so the final
            # scale is T/sum(w1) (host-sent)
            for it in range(ITERS):
                if it == 0:
                    nc.vector.tensor_reduce(kv[:], v3(Kb), axis=AxX, op=Alu.add)
                else:
                    nc.vector.tensor_mul(v3(tb), v3(Kb), bvs)
                    nc.vector.tensor_reduce(kv[:], v3(tb), axis=AxX, op=Alu.add)
                nc.vector.reciprocal(rkv[:], kv[:])
                nc.vector.tensor_mul(us[:, 0:HW], w1, rkv[:])
                if it == ITERS - 1:
                    break
                nc.vector.tensor_mul(v3(tb2), v3(Kt), bus)
                nc.vector.tensor_reduce(kv[:], v3(tb2), axis=AxX, op=Alu.add)
                nc.vector.reciprocal(rkv[:], kv[:])
                nc.vector.tensor_mul(vs[:, 0:HW], w2, rkv[:])

            # ---------------- Phase 3: logits ---------------------------------
            nc.vector.tensor_mul(v3(tb), v3(Kb), bvs)
            bqs = us[:, 0:HW].unsqueeze(2).broadcast_to([BS, HW, HP])
            nc.vector.tensor_mul(v3(tb2), v3(tb), bqs)
            nc.vector.affine_mul_reduce(
                out=jk[:],
                accum_out=lg[:],
                in0=tb2[:],
                in1=flatG[:],
                scale=1.0,
                bias=0.0,
            )
            nc.vector.tensor_mul(lgf[:], lg[:], rs2t)
            nc.sync.dma_start(outp[:, :], lgf[:])


    nc.compile()
    return nc


_NC = None


def _get_nc():
    global _NC
    if _NC is None:
        _NC = build_nc()
    return _NC


def _prep_in_maps(feature_map1, feature_map2):
    q = np.ascontiguousarray(np.asarray(feature_map1, dtype=np.float32)).reshape(
        B_FULL, C, HW
    )
    p = np.ascontiguousarray(np.asarray(feature_map2, dtype=np.float32)).reshape(
        B_FULL, C, HW
    )
    # weight vectors from the RAW features (exact, fp32)
    w1 = np.maximum((q * p.mean(axis=2, keepdims=True)).sum(axis=1), 0.0) + 0.001
    w2 = np.maximum((p * q.mean(axis=2, keepdims=True)).sum(axis=1), 0.0) + 0.001
    rs2t = (TEMP / w1.sum(axis=1, keepdims=True)).astype(np.float32)
    # center + unit-normalize the node vectors (exact, fp32), then fp16
    qc = q - q.mean(axis=1, keepdims=True)
    pc = p - p.mean(axis=1, keepdims=True)
    qn = (qc / (np.linalg.norm(qc, axis=1, keepdims=True) + 1e-8)).astype(np.float16)
    pn = (pc / (np.linalg.norm(pc, axis=1, keepdims=True) + 1e-8)).astype(np.float16)

    in_maps = []
    for i in range(NCORE):
        sl = slice(i * BS, (i + 1) * BS)
        # [NGRP, 128ch, NCH, GRP, 98] -> contiguous group spans
        a = np.zeros((NGRP, 128, WCH), np.float16)
        qi = qn[sl].reshape(NGRP, GRP, NCH, 128, HW).transpose(0, 3, 2, 1, 4)
        pi = pn[sl].reshape(NGRP, GRP, NCH, 128, HW).transpose(0, 3, 2, 1, 4)
        blk = np.empty((NGRP, 128, NCH, GRP, W2), np.float16)
        blk[..., 0:HW] = qi
        blk[..., HW:W2] = pi
        a[:, :, : NCH * JW] = blk.reshape(NGRP, 128, NCH * JW)
        aux = np.empty((BS, HW + HW + 1), np.float32)
        aux[:, 0:HW] = w1[sl]
        aux[:, HW : 2 * HW] = w2[sl]
        aux[:, 2 * HW :] = rs2t[sl]
        in_maps.append({"aug": a, "aux": aux})
    return in_maps


def run(feature_map1, feature_map2, trace=False):
    in_maps = _prep_in_maps(feature_map1, feature_map2)
    nc = _get_nc()
    res = run_bass_kernel_spmd(nc, in_maps, core_ids=list(range(NCORE)), trace=trace)
    out = np.concatenate(
        [np.asarray(res.results[i]["out"]).reshape(BS) for i in range(NCORE)]
    ).astype(np.float32)
    return out, res


def kernel(feature_map1, feature_map2):
    out, _ = run(feature_map1, feature_map2, trace=False)
    return out


# revision 17
# speedup vs baseline: 2.3166x; 1.0442x over previous
"""DeepEMD Trainium2 kernel: batched 49x49 entropic-OT (Sinkhorn) similarity.

Strategy (8 NeuronCores, data-parallel over batch):
- Host prepack does the cheap O(B*C*N) work exactly in fp32: channel-mean
  centering, unit-normalization of the node vectors, the weight vectors
  w1/w2 = relu(<A, mean(B)>)+1e-3 and the final T/sum(w2) scale. The device
  then only needs the cross Gram Q^T P (= the cosine similarity map), exp,
  and the Sinkhorn iterations.
- Each core gets 128 batches. Host packs, per (chunk j of 128 channels,
  batch b), [Qhat | Phat] (128 x 98) in fp16, grouped 16 batches per DMA
  span so loads are contiguous. Stage pool bufs=6 keeps the load stream
  ahead of the PE.
- PE computes sim_b = Qhat^T Phat per batch: 4 accumulating fp16 matmuls
  (weights = Qhat widened to 128 cols for fast-weight-load; junk rows
  49..127 ignored, moving = Phat 49 cols). 8 batches share one PSUM bank
  ([128, 392]); one vector copy per 8 batches moves rows 0..48 to SBUF.
- A per-batch SBUF->SBUF DMA flattens [49, 49] into row b of a [128, 49*50]
  tile (p padded to 50, pad = -5 so exp(pad) == 0); everything after runs
  batch-on-partitions with full 128-lane DVE, 16-bit where possible.
- Sinkhorn runs in the linear domain (K = exp((sim-1)/eps + 16), fp16) with
  Gauss-Seidel updates us = w1/(K vs), vs = w2/(K^T us). K^T is
  materialized by a second (strided-input) ACT exp so both reduce
  directions are contiguous. ITERS=4 gives ~3.5e-3 (tolerance 2e-2).
- logits[b] = T/s2 * us^T ((K.sim) vs) via a fused affine_mul_reduce.
"""

import os
import sys

import numpy as np

sys.path.insert(0, "/opt/trn_rl_repo")

import concourse.bass as bass
import concourse.bacc as bacc
import concourse.mybir as mybir
from concourse import tile
from concourse.bass_utils import run_bass_kernel_spmd

B_FULL, C, HW = 1024, 512, 49
HP = HW + 1  # padded inner dim 50
NCORE = 8
BS = B_FULL // NCORE  # 128 batches per core
NCH = C // 128  # 4 chunks of 128 channels (PE contraction dim)
W2 = 2 * HW  # 98 cols per (batch, chunk): [Qhat | Phat]
GRP = 16  # batches per DMA group
NGRP = BS // GRP
JW = GRP * W2  # 1568 cols per chunk-slab in a group
WCH = NCH * JW + 32  # +32 zero pad so the last 128-wide weight view is legal
FWP = HW * HP  # 2450 padded flat row
ITERS = 4
EPS_S = 0.05
TEMP = 12.5 / HW
EXP_BIAS = 16.0  # K = exp((sim-1)/eps + 16): fp16-safe range [2e-4, 1.35]
PAD = -5.0  # sim pad value; exp((PAD-1)/eps+16) == 0 in fp16

f32 = mybir.dt.float32
f16 = mybir.dt.float16
Alu = mybir.AluOpType
Act = mybir.ActivationFunctionType
AxX = mybir.AxisListType.X


def build_nc(debug=False):
    nc = bacc.Bacc(None, target_bir_lowering=False, debug=debug)
    aug = nc.declare_dram_parameter("aug", [NGRP, 128, WCH], f16, isOutput=False)
    auxp = nc.declare_dram_parameter("aux", [BS, HW + HW + 1], f32, isOutput=False)
    outp = nc.declare_dram_parameter("out", [BS, 1], f32, isOutput=True)

    with tile.TileContext(nc) as tc:
        with (
            tc.tile_pool(name="big", bufs=1) as big,
            tc.tile_pool(name="stage", bufs=8) as stg,
            tc.tile_pool(name="gcopy", bufs=4) as gcp,
            tc.tile_pool(name="small", bufs=1) as sml,
            tc.tile_pool(name="psum", bufs=4, space="PSUM") as pp,
        ):
            # hoist ALL group loads to the very front of the sync queue: the
            # 8 stage bufs hold every group at once, so load issue is never
            # blocked behind flatten issues and tiles never wait on reuse
            ths = []
            NSPL = 4
            SW = WCH // NSPL  # 1576
            for g in range(NGRP):
                th = stg.tile([128, WCH], f16, tag="h", name=f"hg{g}")
                ths.append(th)
                for ss in range(NSPL):
                    nc.sync.dma_start(
                        th[:, ss * SW : (ss + 1) * SW],
                        aug[g, :, ss * SW : (ss + 1) * SW],
                    )

            flatG = big.tile([BS, FWP], f32, tag="flatG", name="flatG")
            nc.vector.memset(flatG[:], PAD)
            aux = sml.tile([BS, HW + HW + 1], f32, tag="aux", name="aux")
            nc.sync.dma_start(aux[:], auxp[:, :])
            w1 = aux[:, 0:HW]
            w2 = aux[:, HW : 2 * HW]
            rs2t = aux[:, 2 * HW : 2 * HW + 1]

            ebias = sml.tile([BS, 1], f32, tag="ebias", name="ebias")
            nc.vector.memset(ebias[:], EXP_BIAS - 1.0 / EPS_S)
            # warm the ACT exp table set early (no data deps -> runs under the
            # phase-1 DMA shadow, hiding the ~2.7us PSEUDO_LOAD_ACT_FUNC_SET)
            wrm = sml.tile([BS, 1], f32, tag="wrm", name="wrm")
            nc.vector.memset(wrm[:], 1.0)
            nc.scalar.activation(wrm[:], wrm[:], Act.Exp)

            # ---------------- Phase 1: cross-Gram + flatten -------------------
            for g in range(NGRP):
                th = ths[g]
                for half in range(2):
                    ps = pp.tile([128, 8 * HW], f32, tag="gram", name="gram")
                    for b8 in range(8):
                        bb = half * 8 + b8
                        for j in range(NCH):
                            woff = j * JW + bb * W2
                            nc.tensor.matmul(
                                ps[:, b8 * HW : (b8 + 1) * HW],
                                th[:, woff : woff + 128],
                                th[:, woff + HW : woff + W2],
                                start=(j == 0),
                                stop=(j == NCH - 1),
                            )
                    st = gcp.tile([HW, 8 * HW], f32, tag="gs", name="gs")
                    nc.vector.tensor_copy(st[:], ps[0:HW, :])
                    for b8 in range(8):
                        b = g * GRP + half * 8 + b8
                        if g < 4:
                            dmae = (nc.gpsimd, nc.scalar)[b % 2]
                        else:
                            # sync's load issues are done by now; recruit it
                            dmae = (nc.gpsimd, nc.scalar, nc.sync)[b % 3]
                        dst = flatG[b : b + 1, :].rearrange(
                            "b (q c) -> b q c", c=HP
                        )[:, :, 0:HW]
                        dmae.dma_start(dst, st[:, b8 * HW : (b8 + 1) * HW])

            # ---------------- Phase 2: K = exp, Sinkhorn (fp16) ---------------
            Kb = big.tile([BS, FWP], f16, tag="K", name="K")  # [b, q, p50]
            Kt = big.tile([BS, FWP], f16, tag="Kt", name="Kt")  # [b, p, q50]
            nc.vector.memset(Kt[:], 0.0)
            tb = big.tile([BS, FWP], f16, tag="tb", name="tb")
            tb2 = big.tile([BS, FWP], f16, tag="tb2", name="tb2")
            jk = big.tile([BS, FWP], f32, tag="jk", name="jk")

            us = sml.tile([BS, HP], f16, tag="us", name="us")
            vs = sml.tile([BS, HP], f16, tag="vs", name="vs")
            nc.vector.memset(us[:], 0.0)
            nc.vector.memset(vs[:], 0.0)
            kv = sml.tile([BS, HW], f32, tag="kv", name="kv")
            rkv = sml.tile([BS, HW], f32, tag="rkv", name="rkv")
            lg = sml.tile([BS, 1], f32, tag="lg", name="lg")
            lgf = sml.tile([BS, 1], f32, tag="lgf", name="lgf")

            def v3(t):  # [128, 49, 50] view
                return t[:].rearrange("p (q c) -> p q c", c=HP)

            # K = exp((sim-1)/eps + 16); pad cols exp(-104) flush to 0 in fp16
            nc.scalar.activation(
                Kb[:], flatG[:], Act.Exp, scale=1.0 / EPS_S, bias=ebias[:]
            )
            # K^T via strided-input exp: in [b, p, q] (inner stride 50),
            # out [b, p, q] contiguous rows of 49 (pad q col stays 0)
            simT = flatG[:].rearrange("b (q c) -> b c q", c=HP)[:, 0:HW, :]
            KtV = v3(Kt)[:, :, 0:HW]
            nc.scalar.activation(KtV, simT, Act.Exp, scale=1.0 / EPS_S, bias=ebias[:])

            bus = us[:].unsqueeze(1).broadcast_to([BS, HW, HP])
            bvs = vs[:].unsqueeze(1).broadcast_to([BS, HW, HP])
            # ends on the u-update: flow row sums = w1 exactly, so the final
            # scale is T/sum(w1) (host-sent)
            for it in range(ITERS):
                if it == 0:
                    nc.vector.tensor_reduce(kv[:], v3(Kb), axis=AxX, op=Alu.add)
                else:
                    nc.vector.tensor_mul(v3(tb), v3(Kb), bvs)
                    nc.vector.tensor_reduce(kv[:], v3(tb), axis=AxX, op=Alu.add)
                nc.vector.reciprocal(rkv[:], kv[:])
                nc.vector.tensor_mul(us[:, 0:HW], w1, rkv[:])
                if it == ITERS - 1:
                    break
                nc.vector.tensor_mul(v3(tb2), v3(Kt), bus)
                nc.vector.tensor_reduce(kv[:], v3(tb2), axis=AxX, op=Alu.add)
                nc.vector.reciprocal(rkv[:], kv[:])
                nc.vector.tensor_mul(vs[:, 0:HW], w2, rkv[:])

            # ---------------- Phase 3: logits ---------------------------------
            nc.vector.tensor_mul(v3(tb), v3(Kb), bvs)
            bqs = us[:, 0:HW].unsqueeze(2).broadcast_to([BS, HW, HP])
            nc.vector.tensor_mul(v3(tb2), v3(tb), bqs)
            nc.vector.affine_mul_reduce(
                out=jk[:],
                accum_out=lg[:],
                in0=tb2[:],
                in1=flatG[:],
                scale=1.0,
                bias=0.0,
            )
            nc.vector.tensor_mul(lgf[:], lg[:], rs2t)
            nc.sync.dma_start(outp[:, :], lgf[:])


    nc.compile()
    return nc


_NC = None


def _get_nc():
    global _NC
    if _NC is None:
        _NC = build_nc()
    return _NC


def _prep_in_maps(feature_map1, feature_map2):
    q = np.ascontiguousarray(np.asarray(feature_map1, dtype=np.float32)).reshape(
        B_FULL, C, HW
    )
    p = np.ascontiguousarray(np.asarray(feature_map2, dtype=np.float32)).reshape(
        B_FULL, C, HW
    )
    # weight vectors from the RAW features (exact, fp32)
    w1 = np.maximum((q * p.mean(axis=2, keepdims=True)).sum(axis=1), 0.0) + 0.001
    w2 = np.maximum((p * q.mean(axis=2, keepdims=True)).sum(axis=1), 0.0) + 0.001
    rs2t = (TEMP / w1.sum(axis=1, keepdims=True)).astype(np.float32)
    # center + unit-normalize the node vectors (exact, fp32), then fp16
    qc = q - q.mean(axis=1, keepdims=True)
    pc = p - p.mean(axis=1, keepdims=True)
    qn = (qc / (np.linalg.norm(qc, axis=1, keepdims=True) + 1e-8)).astype(np.float16)
    pn = (pc / (np.linalg.norm(pc, axis=1, keepdims=True) + 1e-8)).astype(np.float16)

    in_maps = []
    for i in range(NCORE):
        sl = slice(i * BS, (i + 1) * BS)
        # [NGRP, 128ch, NCH, GRP, 98] -> contiguous group spans
        a = np.zeros((NGRP, 128, WCH), np.float16)
        qi = qn[sl].reshape(NGRP, GRP, NCH, 128, HW).transpose(0, 3, 2, 1, 4)
        pi = pn[sl].reshape(NGRP, GRP, NCH, 128, HW).transpose(0, 3, 2, 1, 4)
        blk = np.empty((NGRP, 128, NCH, GRP, W2), np.float16)
        blk[..., 0:HW] = qi
        blk[..., HW:W2] = pi
        a[:, :, : NCH * JW] = blk.reshape(NGRP, 128, NCH * JW)
        aux = np.empty((BS, HW + HW + 1), np.float32)
        aux[:, 0:HW] = w1[sl]
        aux[:, HW : 2 * HW] = w2[sl]
        aux[:, 2 * HW :] = rs2t[sl]
        in_maps.append({"aug": a, "aux": aux})
    return in_maps


def run(feature_map1, feature_map2, trace=False):
    in_maps = _prep_in_maps(feature_map1, feature_map2)
    nc = _get_nc()
    res = run_bass_kernel_spmd(nc, in_maps, core_ids=list(range(NCORE)), trace=trace)
    out = np.concatenate(
        [np.asarray(res.results[i]["out"]).reshape(BS) for i in range(NCORE)]
    ).astype(np.float32)
    return out, res


def kernel(feature_map1, feature_map2):
    out, _ = run(feature_map1, feature_map2, trace=False)
    return out
